# revision 21
# baseline (speedup 1.0000x reference)
"""Trainium2 Bass kernel for nn_MultiHeadedAttention_33835752358170.

Shapes (hardcoded): x [4, 2048, 1024] f32, w_in [192, 1024], b_in [192],
w_out [1024, 64], b_out [1024].  Module quirk: d_k = 64 total across 16
heads -> head_dim = 4.  Scale 1/sqrt(64) = 1/8 folded into q weights.

Sharding: 8 cores = 4 batches x 2 query-halves.  Each core: K/V over the
full S=2048, attention + output projection for its own 1024 query rows.

v2 design (vs baseline):
- exp split between ScalarE (exact Exp) and VectorE (Schraudolph bit-trick:
  int16(round(128*(s*log2e + 127))) bitcast to bf16, ~+-3% multiplicative
  ripple which softmax normalization mostly cancels).  Halves the
  single-engine exp wall that dominated the baseline.
- scores into [128,1536] 3-bank PSUM supertiles (3 strip-half matmuls
  each), one exp instruction per supertile (amortizes engine overhead).
- A@V flipped: the exp-scores tile is the stationary operand (lhsT), rhs a
  [128,5] slice of v_aug ([v_d..., ones]); accumulates attnout in [q, 8h+d]
  layout (denominator at 8h+4) in 2 PSUM banks.  N=5 per matmul instead of
  N=512.
- projections computed compact (64-wide, row order 16s+4d+j), bias folded
  into the PSUM->SBUF copy on ScalarE, strip layout built by one
  SBUF->SBUF scatter DMA per tensor.
- final: reciprocal-normalize, PE transpose, output projection with a bias
  row (b_out + w_out@b_v folded in).
"""

import math

import numpy as np
import ml_dtypes

import concourse.bass as bass
import concourse.mybir as mybir
import concourse.tile as tile
from concourse import bacc
from concourse.bass_utils import run_bass_kernel_spmd

BF16 = ml_dtypes.bfloat16
F32 = np.float32

B, S, DM = 4, 2048, 1024
NH, DK = 16, 64
HD = 4          # head dim
SQ = 1024       # query rows per core
NC_CORES = 8

LOG2E = 1.4426950408889634
EXP_A = 128.0 * LOG2E
EXP_B = 128.0 * 127.0

# per-pass strip-half stream: (j, c, s); two passes over query halves qh
HALVES = [(j, c, s)
          for j in range(4) for c in range(16) for s in range(4)]
N_UNITS = len(HALVES) // 2                # 128 units/pass, 2 halves each
TOT_UNITS = 2 * N_UNITS                   # 256
ACT_UNITS = 137                           # exp units on ScalarE (rest DVE)

_cache = {}


def _unit_on_act(u):
    return (u * ACT_UNITS) // TOT_UNITS != ((u + 1) * ACT_UNITS) // TOT_UNITS


def _build_nc():
    f32 = mybir.dt.float32
    bf16 = mybir.dt.bfloat16
    i16 = mybir.dt.int16
    Exp = mybir.ActivationFunctionType.Exp
    MUL = mybir.AluOpType.mult
    ADD = mybir.AluOpType.add

    nc = bacc.Bacc("TRN2", target_bir_lowering=False, debug=False)

    # ---- DRAM I/O ----
    d_xT = nc.dram_tensor("xT", [DM, S], bf16, kind="ExternalInput").ap()
    d_xqT = nc.dram_tensor("xqT", [DM, SQ], bf16, kind="ExternalInput").ap()
    d_wq = nc.dram_tensor("wq", [DM, DK], bf16, kind="ExternalInput").ap()
    d_wk = nc.dram_tensor("wk", [DM, DK], bf16, kind="ExternalInput").ap()
    d_wv = nc.dram_tensor("wv", [DM, DK], bf16, kind="ExternalInput").ap()
    d_bq = nc.dram_tensor("bq", [DK, 1], f32, kind="ExternalInput").ap()
    d_bk = nc.dram_tensor("bk", [DK, 1], f32, kind="ExternalInput").ap()
    d_wo = nc.dram_tensor("wo", [DK + 1, DM], bf16, kind="ExternalInput").ap()
    d_id = nc.dram_tensor("ident", [128, 128], bf16, kind="ExternalInput").ap()
    d_pl = nc.dram_tensor("plc", [DK, 4, 128], bf16, kind="ExternalInput").ap()
    d_y = nc.dram_tensor("y", [SQ, DM], f32, kind="ExternalOutput").ap()

    with tile.TileContext(nc) as tc:
        with tc.tile_pool(name="const", bufs=1) as cp:
            # ---- load inputs to SBUF ----
            wq_sb = cp.tile([128, 8, DK], bf16)
            wk_sb = cp.tile([128, 8, DK], bf16)
            wv_sb = cp.tile([128, 8, DK], bf16)
            bq_sb = cp.tile([DK, 1], f32)
            bk_sb = cp.tile([DK, 1], f32)
            wo_sb = cp.tile([DK + 1, DM], bf16)
            id_sb = cp.tile([128, 128], bf16)
            xqT_sb = cp.tile([128, 8, SQ], bf16)
            xT_sb = cp.tile([128, 8, S], bf16)

            pl_sb = cp.tile([DK, 4, 128], bf16)
            nc.sync.dma_start(out=pl_sb, in_=d_pl)
            r8 = lambda d: d.rearrange("(k p) n -> p k n", p=128)
            nc.sync.dma_start(out=wq_sb, in_=r8(d_wq))
            nc.sync.dma_start(out=wk_sb, in_=r8(d_wk))
            nc.sync.dma_start(out=wv_sb, in_=r8(d_wv))
            nc.sync.dma_start(out=bq_sb, in_=d_bq)
            nc.sync.dma_start(out=bk_sb, in_=d_bk)
            # per-chunk x loads so the projections pipeline behind the DMA
            for kc in range(8):
                nc.sync.dma_start(out=xqT_sb[:, kc, :],
                                  in_=d_xqT[kc * 128:(kc + 1) * 128, :])
                nc.sync.dma_start(out=xT_sb[:, kc, :],
                                  in_=d_xT[kc * 128:(kc + 1) * 128, :])
            nc.sync.dma_start(out=wo_sb, in_=d_wo)
            nc.sync.dma_start(out=id_sb, in_=d_id)

            # compact projections (row order r = 16s + 4d + j for the
            # scatter below; bias folded into the PSUM->SBUF copy)
            qc_sb = cp.tile([DK, SQ], bf16)
            kc_sb = cp.tile([DK, S], bf16)
            # strip layout: head h=4j+s dim d at partition 32s+d, group j
            qT = cp.tile([128, 4, SQ], bf16)
            kT = cp.tile([128, 4, S], bf16)
            # v_aug: per l-chunk c, head h at cols 8h..8h+4 = [v_0..3, 1]
            va = cp.tile([128, 16, 128], bf16)
            nc.vector.memset(va, 0.0)
            va_h = va.rearrange("p c (h x) -> p c h x", x=8)
            nc.vector.memset(va_h[:, :, :, 4:5], 1.0)
            zlhs = cp.tile([128, 128], bf16)
            nc.vector.memset(zlhs, 0.0)

            # ---- phase 1: projections + strip placement ----
            # ordered so j=0 strips (and low-c v chunks) are ready ASAP
            with tc.tile_pool(name="pp", bufs=1, space="PSUM") as pp, \
                 tc.tile_pool(name="pq", bufs=1, space="PSUM") as pqp:
                qc_ps = pp.tile([DK, SQ], f32, tag="qc")
                for nh in range(2):
                    for kc in range(8):
                        nc.tensor.matmul(
                            qc_ps[:, nh * 512:(nh + 1) * 512],
                            wq_sb[:, kc, :],
                            xqT_sb[:, kc, nh * 512:(nh + 1) * 512],
                            start=(kc == 0), stop=(kc == 7))
                nc.scalar.add(qc_sb, qc_ps, bq_sb)
                kc_ps = pp.tile([DK, S], f32, tag="kc")
                # k compact sh0 on PE while ScalarE copies qc
                for nh in range(2):
                    for kc in range(8):
                        nc.tensor.matmul(
                            kc_ps[:, nh * 512:(nh + 1) * 512],
                            wk_sb[:, kc, :],
                            xT_sb[:, kc, nh * 512:(nh + 1) * 512],
                            start=(kc == 0), stop=(kc == 7))
                nc.scalar.add(kc_sb[:, 0:1024], kc_ps[:, 0:1024], bk_sb)
                # q strip placements (need only qc_sb)
                for j in range(4):
                    qp = pqp.tile([128, SQ], f32, tag="qp")
                    for nh in range(2):
                        nc.tensor.matmul(
                            qp[:, nh * 512:(nh + 1) * 512],
                            pl_sb[:, j, :],
                            qc_sb[:, nh * 512:(nh + 1) * 512],
                            start=True, stop=True)
                    if j % 2 == 0:
                        nc.scalar.copy(qT[:, j, :], qp)
                    else:
                        nc.vector.tensor_copy(qT[:, j, :], qp)
                for nh in range(2):
                    for kc in range(8):
                        nc.tensor.matmul(
                            kc_ps[:, 1024 + nh * 512:1024 + (nh + 1) * 512],
                            wk_sb[:, kc, :],
                            xT_sb[:, kc, 1024 + nh * 512:
                                  1024 + (nh + 1) * 512],
                            start=(kc == 0), stop=(kc == 7))
                nc.vector.tensor_scalar_add(
                    kc_sb[:, 1024:2048], kc_ps[:, 1024:2048], bk_sb)

            with tc.tile_pool(name="pk", bufs=2, space="PSUM") as pkp, \
                 tc.tile_pool(name="pv", bufs=2, space="PSUM") as pvp:
                for sh in range(2):
                    # k strip placements for this half
                    for j in range(4):
                        kp = pkp.tile([128, 1024], f32, tag="kp")
                        for nh in range(2):
                            nc.tensor.matmul(
                                kp[:, nh * 512:(nh + 1) * 512], pl_sb[:, j, :],
                                kc_sb[:, sh * 1024 + nh * 512:
                                      sh * 1024 + (nh + 1) * 512],
                                start=True, stop=True)
                        if j % 2 == 1:
                            nc.scalar.copy(
                                kT[:, j, sh * 1024:(sh + 1) * 1024], kp)
                        else:
                            nc.vector.tensor_copy(
                                kT[:, j, sh * 1024:(sh + 1) * 1024], kp)
                    # v projection for this half -> va slots (4h+d order)
                    for c in range(8 * sh, 8 * sh + 8):
                        pv = pvp.tile([128, DK], f32, tag="pv")
                        for kc in range(8):
                            nc.tensor.matmul(
                                pv, xT_sb[:, kc, c * 128:(c + 1) * 128],
                                wv_sb[:, kc, :],
                                start=(kc == 0), stop=(kc == 7))
                        pv_v = pv.rearrange("p (h d) -> p h d", d=4)
                        nc.vector.tensor_copy(va_h[:, c, :, 0:4], pv_v)

            # ---- phase 2: attention main loop, two query-half passes ----
            # normalized attention output collects in SBUF; the final
            # projection runs as an epilogue once the PSUM pools free up.
            nrm_all = cp.tile([128, 8, 72], bf16)
            nc.vector.memset(nrm_all[:, :, DK:DK + 1], 1.0)
            with tc.tile_pool(name="op", bufs=2, space="PSUM") as op, \
                 tc.tile_pool(name="sp", bufs=3, space="PSUM") as sp, \
                 tc.tile_pool(name="ep", bufs=3) as ep, \
                 tc.tile_pool(name="ns", bufs=2) as nsp:
                for qh in range(2):
                    # attnout: [q(128), ql(4), 8h+d]; denominator at 8h+4
                    outs = op.tile([128, 4, 128], f32, tag="outs")
                    # clear the output bank (zero-weight matmul, start=True;
                    # robust vs stale has_written bits)
                    nc.tensor.matmul(
                        outs.rearrange("p a b -> p (a b)"), zlhs,
                        xqT_sb[:, 0, 0:512], start=True, stop=False)

                    prev = None
                    for u in range(N_UNITS):
                        unit = HALVES[2 * u:2 * u + 2]
                        st = sp.tile([128, 1024], f32, tag="st")
                        et = ep.tile([128, 1024], bf16, tag="et")
                        for i, (j, c, s) in enumerate(unit):
                            nc.tensor.matmul(
                                st[:, i * 512:(i + 1) * 512],
                                kT[32 * s:32 * s + 4, j,
                                   c * 128:(c + 1) * 128],
                                qT[32 * s:32 * s + 4, j,
                                   qh * 512:(qh + 1) * 512],
                                start=True, stop=True,
                                tile_position=(32 * s, 0))
                        if _unit_on_act(qh * N_UNITS + u):
                            nc.scalar.activation(et, st, Exp)
                        else:
                            nc.vector.tensor_scalar(
                                out=et.bitcast(i16), in0=st,
                                scalar1=EXP_A, scalar2=EXP_B,
                                op0=MUL, op1=ADD)
                        if prev is not None:
                            _emit_av(nc, outs, va, prev)
                        prev = (et, unit)
                    _emit_av(nc, outs, va, prev)

                    # normalize into SBUF for this query half
                    for ql in range(4):
                        qc = 4 * qh + ql
                        o_v = outs[:, ql, :].rearrange("p (h x) -> p h x", x=8)
                        rec = nsp.tile([128, NH, 1], f32, tag="rec")
                        nc.vector.reciprocal(rec, o_v[:, :, 4:5])
                        nrm_v = nrm_all[:, qc, 0:DK].rearrange(
                            "p (h d) -> p h d", d=4)
                        nc.vector.tensor_mul(nrm_v, o_v[:, :, 0:4],
                                             rec.broadcast_to([128, NH, 4]))

            # ---- phase 3: transpose + output projection epilogue ----
            with tc.tile_pool(name="fp", bufs=2, space="PSUM") as fp, \
                 tc.tile_pool(name="tpp", bufs=2, space="PSUM") as tpp, \
                 tc.tile_pool(name="fs", bufs=2) as fs:
                for qc in range(8):
                    tp = tpp.tile([DK + 1, 128], bf16, tag="tp")
                    nc.tensor.transpose(tp, nrm_all[:, qc, 0:DK + 1], id_sb)
                    at = fs.tile([DK + 1, 128], bf16, tag="at")
                    nc.scalar.copy(at, tp)
                    yp = fp.tile([128, DM], f32, tag="yp")
                    for nd in range(2):
                        nc.tensor.matmul(
                            yp[:, nd * 512:(nd + 1) * 512], at,
                            wo_sb[:, nd * 512:(nd + 1) * 512],
                            start=True, stop=True)
                    ys = fs.tile([128, DM], f32, tag="ys")
                    if qc % 2 == 0:
                        nc.scalar.copy(ys, yp)
                    else:
                        nc.vector.tensor_copy(ys, yp)
                    nc.sync.dma_start(
                        out=d_y[qc * 128:(qc + 1) * 128, :], in_=ys)

    nc.compile()
    return nc


def _emit_av(nc, outs, va, prev):
    et, unit = prev
    for i, (j, c, s) in enumerate(unit):
        h = 4 * j + s
        for ql in range(4):
            nc.tensor.matmul(
                outs[:, ql, 8 * h:8 * h + 5],
                et[:, i * 512 + ql * 128:i * 512 + (ql + 1) * 128],
                va[:, c, 8 * h:8 * h + 5],
                start=False,
                stop=(j == 3 and c == 15 and s == 3))


def _perm():
    # compact row r = 16s + 4d + j holds (head 4j+s, dim d) = w_in row
    # 4*(4j+s)+d
    p = np.zeros(DK, np.int64)
    for s in range(4):
        for d in range(4):
            for j in range(4):
                p[16 * s + 4 * d + j] = 4 * (4 * j + s) + d
    return p


def _prep_consts(w_in, b_in, w_out, b_out):
    w64 = w_in.astype(np.float64)
    perm = _perm()
    wq = (w64[0:64] / 8.0)[perm].T
    wk = w64[64:128][perm].T
    wv = w64[128:192].T
    bq = (b_in[0:64].astype(np.float64) / 8.0)[perm].reshape(DK, 1)
    bk = b_in[64:128].astype(np.float64)[perm].reshape(DK, 1)
    bv = b_in[128:192].astype(np.float64)

    wo = np.zeros((DK + 1, DM), np.float64)
    wo[0:DK, :] = w_out.astype(np.float64).T        # row 4h+d = w_out[:, 4h+d]
    wo[DK, :] = b_out.astype(np.float64) + w_out.astype(np.float64) @ bv

    plc = np.zeros((DK, 4, 128), np.float64)
    for s in range(4):
        for d in range(4):
            for j in range(4):
                plc[16 * s + 4 * d + j, j, 32 * s + d] = 1.0

    return {
        "wq": wq.astype(BF16), "wk": wk.astype(BF16), "wv": wv.astype(BF16),
        "bq": bq.astype(F32), "bk": bk.astype(F32),
        "wo": wo.astype(BF16), "plc": plc.astype(BF16),
        "ident": np.eye(128, dtype=F32).astype(BF16),
    }


def kernel(x, w_in, b_in, w_out, b_out, _trace=False, **kw):
    x = np.asarray(x, F32)
    consts = _prep_consts(np.asarray(w_in, F32), np.asarray(b_in, F32),
                          np.asarray(w_out, F32), np.asarray(b_out, F32))
    if "nc" not in _cache:
        _cache["nc"] = _build_nc()
    nc = _cache["nc"]

    xTs = [np.ascontiguousarray(x[b].T).astype(BF16) for b in range(B)]
    in_maps = []
    for core in range(NC_CORES):
        b, half = divmod(core, 2)
        m = dict(consts)
        m["xT"] = xTs[b]
        m["xqT"] = np.ascontiguousarray(xTs[b][:, half * SQ:(half + 1) * SQ])
        in_maps.append(m)

    res = run_bass_kernel_spmd(nc, in_maps, list(range(NC_CORES)),
                               trace=_trace)
    out = np.empty((B, S, DM), F32)
    for core in range(NC_CORES):
        b, half = divmod(core, 2)
        out[b, half * SQ:(half + 1) * SQ, :] = res.results[core]["y"]
    if _trace:
        return out, res
    return out
